# revision 25
# baseline (speedup 1.0000x reference)
"""ClusteringAffinity (vq_codebook) Trainium2 kernel — 8 NeuronCores, SPMD.

Math: out[:, :1000] = max over 4 centers of exp(-||f_b - w_{c,j}||^2 / sigma);
out[:, 1000] = rw, a variance-style regularizer over all pairwise center
distances. The mc x mc pairwise matrix is never formed: with
  A = sum_i ||w_i||^2, B = sum_i ||w_i||^4, s = sum_i w_i,
  u = sum_i ||w_i||^2 w_i, G = W^T W  (h x h Gram),
  T = (mc^2 - mc)/2,
  S1 = mc*A - ||s||^2,    S2 = mc*B + A^2 + 2||G||_F^2 - 4 u.s,
  mu = S1/T,              rw = S2/T - mu^2.

Sharding (no collectives -- an 8-rank AllReduce costs ~80us on this stack):
  cores 1..7: distance for 144 classes each (576 centers, zero-padded),
  core 0:     Gram + stats -> rw. One SPMD program, branch on partition id.

v3 notes:
  - core 0 runs the Gram in fp8e4 (w*32; rw rel err ~1e-3 validated on hw)
    with DoubleRow perf mode (2 fp8 rows per PE column cycle; ldweights
    k-tile stride must be %16 == 0, hence the 528-byte row).
  - one accumulation group per PSUM tile: PE start=True zero-fills the
    whole tile, so co-tenant groups lose earlier contributions.
  - s and u come from one extra DoubleRow matmul per pair whose lhsT is
    the (one | q) column pair stored next to the w rows; q = fp8(wsq*2^-5)
    is produced on-device by the wsq pass.
  - all DMAs use partition-major layouts (4-17KB per descriptor): the SP
    sequencer generates descriptors serially at ~5ns each, so row-sized
    descriptors put 3-20us of DMA issue latency on the critical path.
    The contraction rows (h) just need the SAME layout on fT and WT; the
    batch lands device-permuted (d = bc*128+p holds original 4p+bc) and
    the host gather undoes it.
  - distance cores run fp16 end to end (5.4e-3 max rel err on hw).
"""

import numpy as np
import ml_dtypes
from contextlib import ExitStack

B = 512
H = 512
NCLS = 1000
NCEN = 4
SIGMA = 10.0
MC = 4000
MC_PAD = 4096
NPAIR = 16                  # fp8 DoubleRow chunk pairs (256 rows each)
KC = 32                     # 128-row chunks
WROW = 528                  # fp8 row: 512 w + [one, q] + pad; 528 % 16 == 0
WSCALE = 32.0               # w stored as 32*w in fp8 (values ~N(0,2))
QSHIFT = 2.0 ** -10         # q = wsq_raw * 2^-10 = ||w||^2 (~2; stays in the
                            # same magnitude regime as the 32w values, which
                            # the DR weight path demonstrably decodes right)
SHARD_N = 576               # centers per distance core (cores 1..7)
SHARD_C = SHARD_N // NCEN   # 144 classes per distance core
N_CORES = 8
T_PAIRS = (MC * MC - MC) / 2.0
INV_T = 1.0 / T_PAIRS

_CACHE = {}


def _install_tile_patch():
    """walrus on this stack rejects >1 sync-wait on CTRL-class (Drain/NoOp)
    instructions; TileContext's tail drain carries one wait per active proc.
    Emit one SP nop per wait instead."""
    import re
    import concourse.tile as tile
    from bass_rust import ScopedClock, VectorClock

    if getattr(tile.TileContext, "_drain_split_patched", False):
        return

    def _clock_values(vc):
        m = re.search(r"\[([0-9, ]*)\]", repr(vc))
        s = m.group(1).strip()
        return [int(x) for x in s.split(",")] if s else []

    def _patched(self, tick_clock, wait_clock):
        nc = self.nc
        vals = _clock_values(tick_clock.global_clock)
        for i, v in enumerate(vals):
            if v > 0:
                chunk = [0] * len(vals)
                chunk[i] = v
                nop = nc.sync.nop(nofuse=True, hint="tail_wait")
                wait_clock.add_sem_waits(
                    nop.ins, ScopedClock({None: VectorClock(chunk)})
                )
        nc.sync.drain()
        nc.all_engine_barrier()
        assert self.sems is not None
        popped = nc._tile_sem_poison_stack.pop()
        assert popped is self._sem_poison
        nc.clear_and_free_semaphores(list(self.sems.allocated().values()))
        nc.all_engine_barrier()

    tile.TileContext._drain_and_barrier = _patched
    tile.TileContext._drain_split_patched = True


def _install_wait_split_patch():
    """This walrus build accepts at most ONE sync-wait per instruction.
    Rewrite the BIR before compile: hoist excess on_wait entries onto
    same-engine NoOps inserted immediately before the instruction."""
    import json
    import concourse.bass2jax as bass2jax
    import concourse.bass_utils as bass_utils

    if getattr(bass_utils, "_wait_split_patched", False):
        return
    orig = bass_utils.compile_bir_kernel

    # Opcodes with wide sem-update immediate fields; everything else is
    # capped at +1 on this walrus build.
    _WIDE_UPDATE = {"DMACopy", "EventSemaphore", "DMATranspose"}

    def _rewrite(bir_bytes):
        d = json.loads(bir_bytes)
        nid = 0
        changed = False
        for fn in d.get("functions", []):
            for blk in fn.get("blocks", []):
                insts = blk.get("instructions", [])
                new = []
                for inst in insts:
                    si = inst.get("sync_info")
                    waits = (si or {}).get("on_wait") or []
                    if len(waits) > 1:
                        changed = True
                        for w in waits[:-1]:
                            nid += 1
                            new.append({
                                "ins": [],
                                "name": f"WS-{nid}-{inst['name']}",
                                "opcode": "NoOp",
                                "outs": [],
                                "engine": inst["engine"],
                                "sync_info": {"on_update": [], "on_wait": [w]},
                                "text_hint": "wait_split",
                            })
                        si["on_wait"] = [waits[-1]]
                    new.append(inst)
                    ups = (si or {}).get("on_update") or []
                    if (
                        ups
                        and inst.get("opcode") not in _WIDE_UPDATE
                        and any(
                            u.get("update_mode") == "sem-add-imm"
                            and u.get("update_value", 0) > 1
                            for u in ups
                        )
                    ):
                        changed = True
                        keep, hoist = [], []
                        for u in ups:
                            if (
                                u.get("update_mode") == "sem-add-imm"
                                and u.get("update_value", 0) > 1
                            ):
                                hoist.append(u)
                            else:
                                keep.append(u)
                        si["on_update"] = keep
                        # Drain first: an EventSemaphore fires at engine
                        # commit-time, which for PE precedes the PSUM drain —
                        # signalling there would let consumers read stale PSUM.
                        nid += 1
                        new.append({
                            "debug": 0,
                            "ins": [],
                            "is_reset_sema": False,
                            "name": f"DR-{nid}-{inst['name']}",
                            "opcode": "Drain",
                            "outs": [],
                            "engine": inst["engine"],
                            "sync_info": {"on_update": [], "on_wait": []},
                        })
                        for u in hoist:
                            nid += 1
                            new.append({
                                "debug": 0,
                                "ins": [],
                                "name": f"US-{nid}-{inst['name']}",
                                "opcode": "EventSemaphore",
                                "outs": [],
                                "engine": inst["engine"],
                                "sync_info": {"on_update": [u], "on_wait": []},
                            })
                blk["instructions"] = new
        if not changed:
            return bir_bytes
        return json.dumps(d).encode()

    def patched(bir_json, tmpdir, neff_name="file.neff"):
        return orig(_rewrite(bir_json), tmpdir, neff_name=neff_name)

    bass_utils.compile_bir_kernel = patched
    bass2jax.compile_bir_kernel = patched
    bass_utils._wait_split_patched = True


def _build():
    import concourse.bass as bass
    import concourse.tile as tile
    from concourse import mybir

    _install_tile_patch()
    _install_wait_split_patch()

    dt = mybir.dt
    f32 = dt.float32
    f16 = dt.float16
    fp8 = dt.float8e4
    Alu = mybir.AluOpType
    Act = mybir.ActivationFunctionType
    AX = mybir.AxisListType
    DR = mybir.MatmulPerfMode.DoubleRow

    nc = bass.Bass()
    W8 = nc.dram_tensor("W8", [MC_PAD, WROW], fp8, kind="ExternalInput")
    # per-partition: fT rows 4p..4p+3 | WT rows 4p..4p+3 | fR rows 4p..4p+3
    DIN = nc.dram_tensor("DIN", [128, 6400], f16, kind="ExternalInput")
    dist = nc.dram_tensor("dist", [B, SHARD_C], f32, kind="ExternalOutput")
    rw_out = nc.dram_tensor("rw", [1, 1], f32, kind="ExternalOutput")


    with tile.TileContext(nc) as tc, ExitStack() as ctx:
        sb = ctx.enter_context(tc.tile_pool(name="sb", bufs=1))
        wpool = ctx.enter_context(tc.tile_pool(name="wpool", bufs=1))
        spool = ctx.enter_context(tc.tile_pool(name="spool", bufs=3))
        dpool = ctx.enter_context(tc.tile_pool(name="dpool", bufs=2))
        psum = ctx.enter_context(tc.tile_pool(name="psum", bufs=1, space="PSUM"))

        pid = nc.partition_id()
        with tc.If(pid > 0) as cmp:
            # ----- distance branch (cores 1..7), fp16 end-to-end -----
            # Contraction rows land as tile[p, j] = host row 4p+j on BOTH fT
            # and WT (order-consistent, any order works).  The batch lands
            # device-permuted (d = bc*128+p holds original 4p+bc); the host
            # gather undoes it.  Each load: one DMA, 128 contiguous 4-4.6KB
            # descriptors.
            # DIN pieces: P1 = fT(kc0,1)|WT(kc0,1)|fR(bc0) so compute can
            # begin after ~40% of the bytes; P2 = kc2,3 + fR(bc1); P3 = rest.
            din = wpool.tile([128, 6400], f16, tag="din")
            nc.sync.dma_start(din[:, 0:2688], DIN[:, 0:2688])
            nc.sync.dma_start(din[:, 2688:5376], DIN[:, 2688:5376])
            nc.sync.dma_start(din[:, 5376:6400], DIN[:, 5376:6400])
            fTv = [din[:, 0:512], din[:, 512:1024],
                   din[:, 2688:3200], din[:, 3200:3712]]
            WTv = [din[:, 1024:1600], din[:, 1600:2176],
                   din[:, 3712:4288], din[:, 4288:4864]]
            fRv = [din[:, 2176:2688], din[:, 4864:5376],
                   din[:, 5376:5888], din[:, 5888:6400]]

            ones_row = sb.tile([1, 128], f16, tag="ones_row")
            nc.vector.memset(ones_row[:], 1.0)
            neg_half = sb.tile([128, 1], f16, tag="neg_half")
            nc.vector.memset(neg_half[:], -0.5)

            # psum slots: wsq rows allocated first so the dA/dB slots they
            # occupy are reused only after the wrow copies have read them
            wsqA = psum.tile([1, 512], f32, tag="dA", bufs=2, name="wsqA")
            wsqB = psum.tile([1, 64], f32, tag="dB", bufs=1, name="wsqB")
            d_tiles = []
            for bc in range(4):
                dAt = psum.tile([128, 512], f32, tag="dA", bufs=2,
                                name=f"dA{bc}")
                dBt = psum.tile([128, 64], f32, tag="dB", bufs=1,
                                name=f"dB{bc}")
                d_tiles.append((dAt, dBt))

            # fsq: ACT square+accum (3) / DVE mul+reduce (1); bias mults DVE
            bias = sb.tile([128, 4], f32, tag="bias")
            fsq_raw = sb.tile([128, 4], f32, tag="fsq_raw")
            for bc in range(4):
                if bc == 3:
                    fsq_scr = spool.tile([128, H], f16, tag="fsq_v", bufs=1,
                                         name="fsqv")
                    nc.vector.tensor_mul(fsq_scr[:], fRv[bc], fRv[bc])
                    nc.vector.reduce_sum(fsq_raw[:, bc:bc + 1], fsq_scr[:],
                                         axis=AX.X)
                else:
                    fsq_scr = spool.tile([128, H], f16, tag="fsq_a", bufs=2,
                                         name=f"fsqa{bc}")
                    nc.scalar.activation(fsq_scr[:], fRv[bc], Act.Square,
                                         accum_out=fsq_raw[:, bc:bc + 1])
                nc.vector.tensor_scalar_mul(
                    bias[:, bc:bc + 1], fsq_raw[:, bc:bc + 1], -1.0 / SIGMA)

            # WT squares: DVE (kc0,1) / Pool (kc2,3); used by the wsq ones-mm
            sqs = []
            for kc in range(4):
                sq = spool.tile([128, SHARD_N], f16, tag="sq")
                nc.vector.tensor_mul(sq[:], WTv[kc], WTv[kc])
                sqs.append(sq)

            # PE: interleave the wsq ones-matmuls (slot owners) with the
            # early distance matmuls so neither stalls the stream
            dA0, dB0 = d_tiles[0]
            dA1, dB1 = d_tiles[1]
            for kc in range(2):
                nc.tensor.matmul(wsqA[0:1, :], neg_half[:], sqs[kc][:, 0:512],
                                 start=(kc == 0), stop=False,
                                 skip_group_check=True)
            for kc in range(4):
                nc.tensor.matmul(dA0[:], fTv[kc][:, 0:128], WTv[kc][:, 0:512],
                                 start=(kc == 0), stop=False)
            for kc in range(2, 4):
                nc.tensor.matmul(wsqA[0:1, :], neg_half[:], sqs[kc][:, 0:512],
                                 start=False, stop=(kc == 3),
                                 skip_group_check=True)
            for kc in range(4):
                nc.tensor.matmul(wsqB[0:1, :], neg_half[:],
                                 sqs[kc][:, 512:SHARD_N],
                                 start=(kc == 0), stop=(kc == 3))
            dmax_all = dpool.tile([128, 4, SHARD_C], f32, tag="dmax", bufs=1)
            wrow = sb.tile([1, SHARD_N], f16, tag="wrow")
            nc.scalar.copy(wrow[0:1, 0:512], wsqA[0:1, :])
            nc.scalar.copy(wrow[0:1, 512:SHARD_N], wsqB[0:1, :])

            for bc in range(4):
                dA, dB = d_tiles[bc]
                if bc > 0:
                    for kc in range(4):
                        nc.tensor.matmul(
                            dA[:], fTv[kc][:, bc * 128:(bc + 1) * 128],
                            WTv[kc][:, 0:512], start=(kc == 0), stop=False)
                for kc in range(4):
                    nc.tensor.matmul(
                        dB[:], fTv[kc][:, bc * 128:(bc + 1) * 128],
                        WTv[kc][:, 512:SHARD_N], start=(kc == 0), stop=False)
                nc.tensor.matmul(dA[:], ones_row[:],
                                 wrow[0:1, 0:512], start=False, stop=True)
                nc.tensor.matmul(dB[:], ones_row[:],
                                 wrow[0:1, 512:SHARD_N], start=False, stop=True)
                e = dpool.tile([128, SHARD_N], f32, tag="e")
                nc.scalar.activation(e[:, 0:512], dA[:], Act.Exp,
                                     bias=bias[:, bc:bc + 1], scale=2.0 / SIGMA)
                nc.scalar.activation(e[:, 512:SHARD_N], dB[:],
                                     Act.Exp, bias=bias[:, bc:bc + 1],
                                     scale=2.0 / SIGMA)
                nc.vector.reduce_max(
                    dmax_all[:, bc, :],
                    e[:].rearrange("p (c f) -> p c f", f=NCEN),
                    axis=AX.X,
                )
            # device batch bc*128+p is original 4p+bc, so the (p bc) store
            # comes back in natural batch order
            nc.sync.dma_start(
                dist.rearrange("(p bc) c -> p bc c", p=128), dmax_all[:])

        with cmp.Else():
            # ----- Gram + stats branch (core 0), fp8 DoubleRow -----
            # w8_b[p, c, :] = W8 row 32p + c (chunk membership is free, so
            # partition-major rows give 4.2KB descriptors; 4 group-DMAs).
            w8_b = wpool.tile([128, KC, WROW], fp8, tag="w8b")
            W8_r = W8.rearrange("(p c) x -> p c x", c=KC)
            nc.sync.dma_start(w8_b[:, 0:8, :], W8_r[:, 0:8, :])
            nc.sync.dma_start(w8_b[:, 8:16, :], W8_r[:, 8:16, :])
            nc.sync.dma_start(w8_b[:, 16:32, :], W8_r[:, 16:32, :])

            wsq_all = sb.tile([128, KC], f32, tag="wsq_all")
            T0 = psum.tile([128, 512], f32, tag="T0", name="T0")
            T1 = psum.tile([128, 384], f32, tag="T1", name="T1")
            T2 = psum.tile([128, 256], f32, tag="T2", name="T2")
            T3 = psum.tile([128, 128], f32, tag="T3", name="T3")
            SU = psum.tile([2, 512], f32, tag="SU", name="SU")
            # ACT: Square+accum; DVE: fused STT+accum (a Pool square still
            # needs a DVE/ACT reduce that costs as much as doing it all there)
            ENG = [0, 1] * 16
            for g in range(NPAIR):
                for i in range(2):
                    c = 2 * g + i
                    wc = w8_b[:, c, 0:512]
                    e = ENG[c]
                    if e == 0:
                        sq_scr = spool.tile([128, 512], f16, tag="gsqA",
                                            bufs=2, name=f"gsqA{c}")
                        nc.scalar.activation(
                            sq_scr[:], wc, Act.Square,
                            accum_out=wsq_all[:, c:c + 1],
                        )
                    else:
                        sq_scr = spool.tile([128, 512], f16, tag="gsqV",
                                            bufs=2, name=f"gsqV{c}")
                        nc.vector.scalar_tensor_tensor(
                            sq_scr[:], wc, 1.0, wc, Alu.mult, Alu.mult,
                            accum_out=wsq_all[:, c:c + 1],
                        )
                nc.vector.tensor_scalar_mul(
                    w8_b[:, 2 * g:2 * g + 2, 513:514],
                    wsq_all[:, 2 * g:2 * g + 2].rearrange("p (i x) -> p i x", x=1),
                    QSHIFT,
                )
                st = (g == 0)
                sp = (g == NPAIR - 1)
                pr = w8_b[:, 2 * g:2 * g + 2, :]
                nc.tensor.matmul(T0[:], pr[:, :, 0:128], pr[:, :, 0:512],
                                 start=st, stop=sp, perf_mode=DR)
                nc.tensor.matmul(T1[:], pr[:, :, 128:256], pr[:, :, 128:512],
                                 start=st, stop=sp, perf_mode=DR)
                nc.tensor.matmul(T2[:], pr[:, :, 256:384], pr[:, :, 256:512],
                                 start=st, stop=sp, perf_mode=DR)
                nc.tensor.matmul(T3[:], pr[:, :, 384:512], pr[:, :, 384:512],
                                 start=st, stop=sp, perf_mode=DR)
                nc.tensor.matmul(SU[:], pr[:, :, 512:514], pr[:, :, 0:512],
                                 start=st, stop=sp, perf_mode=DR)

            # ---- endgame ----
            # stats_cols: [g2 T0d, T0o, T1d, T1o, T2d, T2o, T3d, A, B]
            stats_cols = sb.tile([128, 9], f32, tag="stats_cols")
            IS = 1.0 / (WSCALE * WSCALE)  # undo psum scale 1024 on G
            SQ2 = 1.4142135623730951
            junk = spool.tile([128, 512], f16, tag="junk")
            nc.scalar.activation(junk[:, 0:128], T0[:, 0:128], Act.Square,
                                 scale=IS, accum_out=stats_cols[:, 0:1])
            nc.scalar.activation(junk[:, 0:384], T0[:, 128:512], Act.Square,
                                 scale=SQ2 * IS, accum_out=stats_cols[:, 1:2])
            nc.scalar.activation(junk[:, 0:128], T1[:, 0:128], Act.Square,
                                 scale=IS, accum_out=stats_cols[:, 2:3])
            nc.scalar.activation(junk[:, 0:256], T1[:, 128:384], Act.Square,
                                 scale=SQ2 * IS, accum_out=stats_cols[:, 3:4])
            # T2/T3 squares on the DVE pipe (psum -> sbuf copy, then STT)
            t2c = spool.tile([128, 256], f32, tag="t2c")
            nc.vector.tensor_copy(t2c[:], T2[:])
            jk2 = spool.tile([128, 256], f32, tag="jk2")
            nc.vector.scalar_tensor_tensor(
                jk2[:, 0:128], t2c[:, 0:128], IS * IS, t2c[:, 0:128],
                Alu.mult, Alu.mult, accum_out=stats_cols[:, 4:5])
            nc.vector.scalar_tensor_tensor(
                jk2[:, 128:256], t2c[:, 128:256], 2.0 * IS * IS, t2c[:, 128:256],
                Alu.mult, Alu.mult, accum_out=stats_cols[:, 5:6])
            t3c = spool.tile([128, 128], f32, tag="t3c")
            nc.vector.tensor_copy(t3c[:], T3[:])
            jk3 = spool.tile([128, 128], f32, tag="jk3")
            nc.vector.scalar_tensor_tensor(
                jk3[:], t3c[:], IS * IS, t3c[:],
                Alu.mult, Alu.mult, accum_out=stats_cols[:, 6:7])
            nc.vector.reduce_sum(stats_cols[:, 7:8], wsq_all[:], axis=AX.X)
            wsq_scr = spool.tile([128, KC], f32, tag="wsq_scr")
            nc.vector.scalar_tensor_tensor(
                wsq_scr[:], wsq_all[:], 1.0, wsq_all[:],
                Alu.mult, Alu.mult, accum_out=stats_cols[:, 8:9],
            )
            # s, u rows -> SBUF; u to partition 0 via DMA; dots into scr
            scr = sb.tile([1, 16], f32, tag="scr")
            su_row = sb.tile([2, 512], f32, tag="su_row")
            nc.vector.tensor_copy(su_row[:], SU[:])
            u_row = sb.tile([1, 512], f32, tag="u_row")
            nc.sync.dma_start(u_row[0:1, :], su_row[1:2, :])
            su_junk = spool.tile([1, 512], f32, tag="su_junk")
            nc.vector.scalar_tensor_tensor(
                su_junk[0:1, :], su_row[0:1, :], 1.0, su_row[0:1, :],
                Alu.mult, Alu.mult, accum_out=scr[0:1, 12:13],
            )
            nc.vector.scalar_tensor_tensor(
                su_junk[0:1, :], su_row[0:1, :], 1.0, u_row[0:1, :],
                Alu.mult, Alu.mult, accum_out=scr[0:1, 13:14],
            )
            # cross-partition reduce of the 9 partials in one fp32 ones-matmul
            ones_col = sb.tile([128, 1], f32, tag="ones_col")
            nc.vector.memset(ones_col[:], 1.0)
            cpres = T3[0:1, 16:25]
            nc.tensor.matmul(cpres, ones_col[:], stats_cols[:],
                             start=True, stop=True, skip_group_check=True)

            # scalar assembly on partition 0.  Raw scales:
            #   A_cp = 2^10 A, B_cp = 2^20 B, ssq_cp = 2^10 ssq,
            #   us_cp = 2^10 us (q = wsq, u_row = 32u), g2 = true ||G||^2.
            nc.vector.tensor_copy(scr[0:1, 0:9], cpres)
            g2s = scr[0:1, 9:10]
            nc.vector.reduce_sum(g2s, scr[0:1, 0:7], axis=AX.X)
            A_ap = scr[0:1, 7:8]
            B_ap = scr[0:1, 8:9]
            ssq_ap = scr[0:1, 12:13]
            us_ap = scr[0:1, 13:14]
            x_ap = scr[0:1, 10:11]
            t_ap = scr[0:1, 11:12]
            rw_ap = scr[0:1, 14:15]
            # X = 2^10*S1 = mc*A_cp - ssq_cp
            nc.vector.scalar_tensor_tensor(
                x_ap, A_ap, float(MC), ssq_ap, Alu.mult, Alu.subtract)
            # t = (X * 2^-10/T)^2 = mu^2
            nc.vector.tensor_scalar_mul(x_ap, x_ap, INV_T / 1024.0)
            nc.vector.tensor_mul(t_ap, x_ap, x_ap)
            # Y = 2^20*S2 = mc*B_cp + A_cp^2 + 2^21*g2 - 2^7*us_cp
            a2 = scr[0:1, 10:11]  # reuse
            nc.vector.tensor_mul(a2, A_ap, A_ap)
            nc.vector.scalar_tensor_tensor(
                rw_ap, B_ap, float(MC), a2, Alu.mult, Alu.add)
            nc.vector.tensor_scalar_mul(us_ap, us_ap, 4096.0)  # 2^12*us_cp
            nc.vector.scalar_tensor_tensor(
                a2, g2s, float(2.0 ** 21), us_ap, Alu.mult, Alu.subtract)
            nc.vector.tensor_add(rw_ap, rw_ap, a2)
            # rw = Y*2^-20/T - mu^2
            nc.vector.scalar_tensor_tensor(
                rw_ap, rw_ap, INV_T / (2.0 ** 20), t_ap, Alu.mult, Alu.subtract)
            nc.sync.dma_start(rw_out[0:1, 0:1], rw_ap)

    return nc


def _batch_perm():
    # device batch index d = bc*128 + p holds original row 4p + bc
    d = np.arange(B)
    p, bc = d % 128, d // 128
    return 4 * p + bc


def _prep_inputs(f, W):
    e4 = ml_dtypes.float8_e4m3
    f16 = np.float16
    f = np.asarray(f, dtype=np.float32)
    w_flat = np.asarray(W, dtype=np.float32).reshape(MC, H)

    W8_full = np.zeros((MC_PAD, WROW), dtype=e4)
    W8_full[:MC, :H] = (w_flat * WSCALE).astype(e4)
    W8_full[:, H] = e4(1.0)

    operm = _batch_perm()
    fh = f.astype(f16)
    fT_np = np.ascontiguousarray(fh[operm].T)  # column d = original 4p+bc
    fR_np = fh                                 # natural: (p bc) rows
    Wh = w_flat.astype(f16)
    z_W8 = np.zeros((MC_PAD, WROW), dtype=e4)
    z_DIN = np.zeros((128, 6400), dtype=f16)

    in_maps = [{"W8": W8_full, "DIN": z_DIN}]
    for k in range(1, N_CORES):
        r0 = SHARD_N * (k - 1)
        r1 = min(r0 + SHARD_N, MC)
        WT_np = np.zeros((H, SHARD_N), dtype=f16)
        WT_np[:, : r1 - r0] = Wh[r0:r1].T
        fT4 = fT_np.reshape(128, 4, B)
        WT4 = WT_np.reshape(128, 4, SHARD_N)
        fR4 = fR_np.reshape(128, 4, H)
        din = np.concatenate([
            fT4[:, 0:2].reshape(128, -1), WT4[:, 0:2].reshape(128, -1),
            fR4[:, 0].reshape(128, -1),
            fT4[:, 2:4].reshape(128, -1), WT4[:, 2:4].reshape(128, -1),
            fR4[:, 1].reshape(128, -1),
            fR4[:, 2:4].reshape(128, -1),
        ], axis=1)
        in_maps.append({"W8": z_W8, "DIN": np.ascontiguousarray(din)})
    return in_maps


def kernel(f, W, trace=False):
    from concourse.bass_utils import run_bass_kernel_spmd

    nc = _CACHE.get("nc")
    if nc is None:
        nc = _build()
        _CACHE["nc"] = nc

    in_maps = _prep_inputs(f, W)
    kwargs = {}
    if trace:
        kwargs["trace_cores"] = [0, 4]
    res = run_bass_kernel_spmd(
        nc, in_maps, core_ids=list(range(N_CORES)), trace=trace, **kwargs
    )
    _CACHE["last_result"] = res

    out = np.empty((B, NCLS + 1), dtype=np.float32)
    for k in range(1, N_CORES):
        c0 = SHARD_C * (k - 1)
        ncls = min(SHARD_C, NCLS - c0)
        if ncls <= 0:
            continue
        out[:, c0:c0 + ncls] = res.results[k]["dist"][:, :ncls]
    out[:, NCLS] = res.results[0]["rw"][0, 0]
    return out


# revision 26
# speedup vs baseline: 1.0139x; 1.0139x over previous
"""ClusteringAffinity (vq_codebook) Trainium2 kernel — 8 NeuronCores, SPMD.

Math: out[:, :1000] = max over 4 centers of exp(-||f_b - w_{c,j}||^2 / sigma);
out[:, 1000] = rw, a variance-style regularizer over all pairwise center
distances. The mc x mc pairwise matrix is never formed: with
  A = sum_i ||w_i||^2, B = sum_i ||w_i||^4, s = sum_i w_i,
  u = sum_i ||w_i||^2 w_i, G = W^T W  (h x h Gram),
  T = (mc^2 - mc)/2,
  S1 = mc*A - ||s||^2,    S2 = mc*B + A^2 + 2||G||_F^2 - 4 u.s,
  mu = S1/T,              rw = S2/T - mu^2.

Sharding (no collectives -- an 8-rank AllReduce costs ~80us on this stack):
  cores 1..7: distance for 144 classes each (576 centers, zero-padded),
  core 0:     Gram + stats -> rw. One SPMD program, branch on partition id.

v3 notes:
  - core 0 runs the Gram in fp8e4 (w*32; rw rel err ~1e-3 validated on hw)
    with DoubleRow perf mode (2 fp8 rows per PE column cycle; ldweights
    k-tile stride must be %16 == 0, hence the 528-byte row).
  - one accumulation group per PSUM tile: PE start=True zero-fills the
    whole tile, so co-tenant groups lose earlier contributions.
  - s and u come from one extra DoubleRow matmul per pair whose lhsT is
    the (one | q) column pair stored next to the w rows; q = fp8(wsq*2^-5)
    is produced on-device by the wsq pass.
  - all DMAs use partition-major layouts (4-17KB per descriptor): the SP
    sequencer generates descriptors serially at ~5ns each, so row-sized
    descriptors put 3-20us of DMA issue latency on the critical path.
    The contraction rows (h) just need the SAME layout on fT and WT; the
    batch lands device-permuted (d = bc*128+p holds original 4p+bc) and
    the host gather undoes it.
  - distance cores run fp16 end to end (5.4e-3 max rel err on hw).
"""

import numpy as np
import ml_dtypes
from contextlib import ExitStack

B = 512
H = 512
NCLS = 1000
NCEN = 4
SIGMA = 10.0
MC = 4000
MC_PAD = 4096
NPAIR = 16                  # fp8 DoubleRow chunk pairs (256 rows each)
KC = 32                     # 128-row chunks
WROW = 528                  # fp8 row: 512 w + [one, q] + pad; 528 % 16 == 0
WSCALE = 32.0               # w stored as 32*w in fp8 (values ~N(0,2))
QSHIFT = 2.0 ** -10         # q = wsq_raw * 2^-10 = ||w||^2 (~2; stays in the
                            # same magnitude regime as the 32w values, which
                            # the DR weight path demonstrably decodes right)
SHARD_N = 576               # centers per distance core (cores 1..7)
SHARD_C = SHARD_N // NCEN   # 144 classes per distance core
N_CORES = 8
T_PAIRS = (MC * MC - MC) / 2.0
INV_T = 1.0 / T_PAIRS

_CACHE = {}


def _install_tile_patch():
    """walrus on this stack rejects >1 sync-wait on CTRL-class (Drain/NoOp)
    instructions; TileContext's tail drain carries one wait per active proc.
    Emit one SP nop per wait instead."""
    import re
    import concourse.tile as tile
    from bass_rust import ScopedClock, VectorClock

    if getattr(tile.TileContext, "_drain_split_patched", False):
        return

    def _clock_values(vc):
        m = re.search(r"\[([0-9, ]*)\]", repr(vc))
        s = m.group(1).strip()
        return [int(x) for x in s.split(",")] if s else []

    def _patched(self, tick_clock, wait_clock):
        nc = self.nc
        vals = _clock_values(tick_clock.global_clock)
        for i, v in enumerate(vals):
            if v > 0:
                chunk = [0] * len(vals)
                chunk[i] = v
                nop = nc.sync.nop(nofuse=True, hint="tail_wait")
                wait_clock.add_sem_waits(
                    nop.ins, ScopedClock({None: VectorClock(chunk)})
                )
        nc.sync.drain()
        nc.all_engine_barrier()
        assert self.sems is not None
        popped = nc._tile_sem_poison_stack.pop()
        assert popped is self._sem_poison
        nc.clear_and_free_semaphores(list(self.sems.allocated().values()))
        nc.all_engine_barrier()

    tile.TileContext._drain_and_barrier = _patched
    tile.TileContext._drain_split_patched = True


def _install_wait_split_patch():
    """This walrus build accepts at most ONE sync-wait per instruction.
    Rewrite the BIR before compile: hoist excess on_wait entries onto
    same-engine NoOps inserted immediately before the instruction."""
    import json
    import concourse.bass2jax as bass2jax
    import concourse.bass_utils as bass_utils

    if getattr(bass_utils, "_wait_split_patched", False):
        return
    orig = bass_utils.compile_bir_kernel

    # Opcodes with wide sem-update immediate fields; everything else is
    # capped at +1 on this walrus build.
    _WIDE_UPDATE = {"DMACopy", "EventSemaphore", "DMATranspose"}

    def _rewrite(bir_bytes):
        d = json.loads(bir_bytes)
        nid = 0
        changed = False
        for fn in d.get("functions", []):
            for blk in fn.get("blocks", []):
                insts = blk.get("instructions", [])
                new = []
                for inst in insts:
                    si = inst.get("sync_info")
                    waits = (si or {}).get("on_wait") or []
                    if len(waits) > 1:
                        changed = True
                        for w in waits[:-1]:
                            nid += 1
                            new.append({
                                "ins": [],
                                "name": f"WS-{nid}-{inst['name']}",
                                "opcode": "NoOp",
                                "outs": [],
                                "engine": inst["engine"],
                                "sync_info": {"on_update": [], "on_wait": [w]},
                                "text_hint": "wait_split",
                            })
                        si["on_wait"] = [waits[-1]]
                    new.append(inst)
                    ups = (si or {}).get("on_update") or []
                    if (
                        ups
                        and inst.get("opcode") not in _WIDE_UPDATE
                        and any(
                            u.get("update_mode") == "sem-add-imm"
                            and u.get("update_value", 0) > 1
                            for u in ups
                        )
                    ):
                        changed = True
                        keep, hoist = [], []
                        for u in ups:
                            if (
                                u.get("update_mode") == "sem-add-imm"
                                and u.get("update_value", 0) > 1
                            ):
                                hoist.append(u)
                            else:
                                keep.append(u)
                        si["on_update"] = keep
                        # Drain first: an EventSemaphore fires at engine
                        # commit-time, which for PE precedes the PSUM drain —
                        # signalling there would let consumers read stale PSUM.
                        nid += 1
                        new.append({
                            "debug": 0,
                            "ins": [],
                            "is_reset_sema": False,
                            "name": f"DR-{nid}-{inst['name']}",
                            "opcode": "Drain",
                            "outs": [],
                            "engine": inst["engine"],
                            "sync_info": {"on_update": [], "on_wait": []},
                        })
                        for u in hoist:
                            nid += 1
                            new.append({
                                "debug": 0,
                                "ins": [],
                                "name": f"US-{nid}-{inst['name']}",
                                "opcode": "EventSemaphore",
                                "outs": [],
                                "engine": inst["engine"],
                                "sync_info": {"on_update": [u], "on_wait": []},
                            })
                blk["instructions"] = new
        if not changed:
            return bir_bytes
        return json.dumps(d).encode()

    def patched(bir_json, tmpdir, neff_name="file.neff"):
        return orig(_rewrite(bir_json), tmpdir, neff_name=neff_name)

    bass_utils.compile_bir_kernel = patched
    bass2jax.compile_bir_kernel = patched
    bass_utils._wait_split_patched = True


def _build():
    import concourse.bass as bass
    import concourse.tile as tile
    from concourse import mybir

    _install_tile_patch()
    _install_wait_split_patch()

    dt = mybir.dt
    f32 = dt.float32
    f16 = dt.float16
    fp8 = dt.float8e4
    Alu = mybir.AluOpType
    Act = mybir.ActivationFunctionType
    AX = mybir.AxisListType
    DR = mybir.MatmulPerfMode.DoubleRow

    nc = bass.Bass()
    W8 = nc.dram_tensor("W8", [MC_PAD, WROW], fp8, kind="ExternalInput")
    # per-partition: fT rows 4p..4p+3 | WT rows 4p..4p+3 | fR rows 4p..4p+3
    DIN = nc.dram_tensor("DIN", [128, 6400], f16, kind="ExternalInput")
    dist = nc.dram_tensor("dist", [B, SHARD_C], f32, kind="ExternalOutput")
    rw_out = nc.dram_tensor("rw", [1, 1], f32, kind="ExternalOutput")


    with tile.TileContext(nc) as tc, ExitStack() as ctx:
        sb = ctx.enter_context(tc.tile_pool(name="sb", bufs=1))
        wpool = ctx.enter_context(tc.tile_pool(name="wpool", bufs=1))
        spool = ctx.enter_context(tc.tile_pool(name="spool", bufs=3))
        dpool = ctx.enter_context(tc.tile_pool(name="dpool", bufs=2))
        psum = ctx.enter_context(tc.tile_pool(name="psum", bufs=1, space="PSUM"))

        pid = nc.partition_id()
        with tc.If(pid > 0) as cmp:
            # ----- distance branch (cores 1..7), fp16 end-to-end -----
            # Contraction rows land as tile[p, j] = host row 4p+j on BOTH fT
            # and WT (order-consistent, any order works).  The batch lands
            # device-permuted (d = bc*128+p holds original 4p+bc); the host
            # gather undoes it.  Each load: one DMA, 128 contiguous 4-4.6KB
            # descriptors.
            # DIN pieces: P1 = fT(kc0,1)|WT(kc0,1)|fR(bc0) so compute can
            # begin after ~40% of the bytes; P2 = kc2,3 + fR(bc1); P3 = rest.
            din = wpool.tile([128, 6400], f16, tag="din")
            nc.sync.dma_start(din[:, 0:2688], DIN[:, 0:2688])
            nc.sync.dma_start(din[:, 2688:5376], DIN[:, 2688:5376])
            nc.sync.dma_start(din[:, 5376:6400], DIN[:, 5376:6400])
            fTv = [din[:, 0:512], din[:, 512:1024],
                   din[:, 2688:3200], din[:, 3200:3712]]
            WTv = [din[:, 1024:1600], din[:, 1600:2176],
                   din[:, 3712:4288], din[:, 4288:4864]]
            fRv = [din[:, 2176:2688], din[:, 4864:5376],
                   din[:, 5376:5888], din[:, 5888:6400]]

            ones_row = sb.tile([1, 128], f16, tag="ones_row")
            nc.vector.memset(ones_row[:], 1.0)
            neg_half = sb.tile([128, 1], f16, tag="neg_half")
            nc.vector.memset(neg_half[:], -0.5)

            # psum slots: wsq rows allocated first so the dA/dB slots they
            # occupy are reused only after the wrow copies have read them
            wsqA = psum.tile([1, 512], f32, tag="dA", bufs=2, name="wsqA")
            wsqB = psum.tile([1, 64], f32, tag="dB", bufs=1, name="wsqB")
            d_tiles = []
            for bc in range(4):
                dAt = psum.tile([128, 512], f32, tag="dA", bufs=2,
                                name=f"dA{bc}")
                dBt = psum.tile([128, 64], f32, tag="dB", bufs=1,
                                name=f"dB{bc}")
                d_tiles.append((dAt, dBt))

            # fsq: ACT square+accum (3) / DVE mul+reduce (1); bias mults DVE
            bias = sb.tile([128, 4], f32, tag="bias")
            fsq_raw = sb.tile([128, 4], f32, tag="fsq_raw")
            for bc in range(4):
                if bc == 3:
                    fsq_scr = spool.tile([128, H], f16, tag="fsq_v", bufs=1,
                                         name="fsqv")
                    nc.vector.tensor_mul(fsq_scr[:], fRv[bc], fRv[bc])
                    nc.vector.reduce_sum(fsq_raw[:, bc:bc + 1], fsq_scr[:],
                                         axis=AX.X)
                else:
                    fsq_scr = spool.tile([128, H], f16, tag="fsq_a", bufs=2,
                                         name=f"fsqa{bc}")
                    nc.scalar.activation(fsq_scr[:], fRv[bc], Act.Square,
                                         accum_out=fsq_raw[:, bc:bc + 1])
                nc.vector.tensor_scalar_mul(
                    bias[:, bc:bc + 1], fsq_raw[:, bc:bc + 1], -1.0 / SIGMA)

            # WT squares: DVE (kc0,1) / Pool (kc2,3); used by the wsq ones-mm
            sqs = []
            for kc in range(4):
                sq = spool.tile([128, SHARD_N], f16, tag="sq")
                nc.vector.tensor_mul(sq[:], WTv[kc], WTv[kc])
                sqs.append(sq)

            # PE: interleave the wsq ones-matmuls (slot owners) with the
            # early distance matmuls so neither stalls the stream
            dA0, dB0 = d_tiles[0]
            dA1, dB1 = d_tiles[1]
            for kc in range(2):
                nc.tensor.matmul(wsqA[0:1, :], neg_half[:], sqs[kc][:, 0:512],
                                 start=(kc == 0), stop=False,
                                 skip_group_check=True)
            for kc in range(4):
                nc.tensor.matmul(dA0[:], fTv[kc][:, 0:128], WTv[kc][:, 0:512],
                                 start=(kc == 0), stop=False)
            for kc in range(2, 4):
                nc.tensor.matmul(wsqA[0:1, :], neg_half[:], sqs[kc][:, 0:512],
                                 start=False, stop=(kc == 3),
                                 skip_group_check=True)
            for kc in range(4):
                nc.tensor.matmul(wsqB[0:1, :], neg_half[:],
                                 sqs[kc][:, 512:SHARD_N],
                                 start=(kc == 0), stop=(kc == 3))
            dmax_all = dpool.tile([128, 4, SHARD_C], f32, tag="dmax", bufs=1)
            wrow = sb.tile([1, SHARD_N], f16, tag="wrow")
            nc.scalar.copy(wrow[0:1, 0:512], wsqA[0:1, :])
            nc.scalar.copy(wrow[0:1, 512:SHARD_N], wsqB[0:1, :])

            for bc in range(4):
                dA, dB = d_tiles[bc]
                for kc in range(4):
                    nc.tensor.matmul(
                        dB[:], fTv[kc][:, bc * 128:(bc + 1) * 128],
                        WTv[kc][:, 512:SHARD_N], start=(kc == 0), stop=False)
                if bc < 3:
                    dAn, _ = d_tiles[bc + 1]
                    for kc in range(4):
                        nc.tensor.matmul(
                            dAn[:], fTv[kc][:, (bc + 1) * 128:(bc + 2) * 128],
                            WTv[kc][:, 0:512], start=(kc == 0), stop=False)
                nc.tensor.matmul(dA[:], ones_row[:],
                                 wrow[0:1, 0:512], start=False, stop=True)
                nc.tensor.matmul(dB[:], ones_row[:],
                                 wrow[0:1, 512:SHARD_N], start=False, stop=True)
                e = dpool.tile([128, SHARD_N], f32, tag="e")
                nc.scalar.activation(e[:, 0:512], dA[:], Act.Exp,
                                     bias=bias[:, bc:bc + 1], scale=2.0 / SIGMA)
                nc.scalar.activation(e[:, 512:SHARD_N], dB[:],
                                     Act.Exp, bias=bias[:, bc:bc + 1],
                                     scale=2.0 / SIGMA)
                nc.vector.reduce_max(
                    dmax_all[:, bc, :],
                    e[:].rearrange("p (c f) -> p c f", f=NCEN),
                    axis=AX.X,
                )
            # device batch bc*128+p is original 4p+bc, so the (p bc) store
            # comes back in natural batch order
            nc.sync.dma_start(
                dist.rearrange("(p bc) c -> p bc c", p=128), dmax_all[:])

        with cmp.Else():
            # ----- Gram + stats branch (core 0), fp8 DoubleRow -----
            # w8_b[p, c, :] = W8 row 32p + c (chunk membership is free, so
            # partition-major rows give 4.2KB descriptors; 4 group-DMAs).
            w8_b = wpool.tile([128, KC, WROW], fp8, tag="w8b")
            W8_r = W8.rearrange("(p c) x -> p c x", c=KC)
            nc.sync.dma_start(w8_b[:, 0:8, :], W8_r[:, 0:8, :])
            nc.sync.dma_start(w8_b[:, 8:16, :], W8_r[:, 8:16, :])
            nc.sync.dma_start(w8_b[:, 16:32, :], W8_r[:, 16:32, :])

            wsq_all = sb.tile([128, KC], f32, tag="wsq_all")
            T0 = psum.tile([128, 512], f32, tag="T0", name="T0")
            T1 = psum.tile([128, 384], f32, tag="T1", name="T1")
            T2 = psum.tile([128, 256], f32, tag="T2", name="T2")
            T3 = psum.tile([128, 128], f32, tag="T3", name="T3")
            SU = psum.tile([2, 512], f32, tag="SU", name="SU")
            # ACT: Square+accum; DVE: fused STT+accum (a Pool square still
            # needs a DVE/ACT reduce that costs as much as doing it all there)
            ENG = [0, 1, 0, 1, 0, 1, 0, 1, 0, 1, 0, 1, 0, 1, 1, 0,
                   1, 0, 1, 0, 1, 0, 1, 0, 1, 0, 1, 1, 0, 1, 0, 1]
            for g in range(NPAIR):
                for i in range(2):
                    c = 2 * g + i
                    wc = w8_b[:, c, 0:512]
                    e = ENG[c]
                    if e == 0:
                        sq_scr = spool.tile([128, 512], f16, tag="gsqA",
                                            bufs=2, name=f"gsqA{c}")
                        nc.scalar.activation(
                            sq_scr[:], wc, Act.Square,
                            accum_out=wsq_all[:, c:c + 1],
                        )
                    else:
                        sq_scr = spool.tile([128, 512], f16, tag="gsqV",
                                            bufs=2, name=f"gsqV{c}")
                        nc.vector.scalar_tensor_tensor(
                            sq_scr[:], wc, 1.0, wc, Alu.mult, Alu.mult,
                            accum_out=wsq_all[:, c:c + 1],
                        )
                nc.vector.tensor_scalar_mul(
                    w8_b[:, 2 * g:2 * g + 2, 513:514],
                    wsq_all[:, 2 * g:2 * g + 2].rearrange("p (i x) -> p i x", x=1),
                    QSHIFT,
                )
                st = (g == 0)
                sp = (g == NPAIR - 1)
                pr = w8_b[:, 2 * g:2 * g + 2, :]
                nc.tensor.matmul(T0[:], pr[:, :, 0:128], pr[:, :, 0:512],
                                 start=st, stop=sp, perf_mode=DR)
                nc.tensor.matmul(T1[:], pr[:, :, 128:256], pr[:, :, 128:512],
                                 start=st, stop=sp, perf_mode=DR)
                nc.tensor.matmul(T2[:], pr[:, :, 256:384], pr[:, :, 256:512],
                                 start=st, stop=sp, perf_mode=DR)
                nc.tensor.matmul(T3[:], pr[:, :, 384:512], pr[:, :, 384:512],
                                 start=st, stop=sp, perf_mode=DR)
                nc.tensor.matmul(SU[:], pr[:, :, 512:514], pr[:, :, 0:512],
                                 start=st, stop=sp, perf_mode=DR)

            # ---- endgame ----
            # stats_cols: [g2 T0d, T0o, T1d, T1o, T2d, T2o, T3d, A, B]
            stats_cols = sb.tile([128, 9], f32, tag="stats_cols")
            IS = 1.0 / (WSCALE * WSCALE)  # undo psum scale 1024 on G
            SQ2 = 1.4142135623730951
            junk = spool.tile([128, 512], f16, tag="junk")
            nc.scalar.activation(junk[:, 0:128], T0[:, 0:128], Act.Square,
                                 scale=IS, accum_out=stats_cols[:, 0:1])
            nc.scalar.activation(junk[:, 0:384], T0[:, 128:512], Act.Square,
                                 scale=SQ2 * IS, accum_out=stats_cols[:, 1:2])
            nc.scalar.activation(junk[:, 0:128], T1[:, 0:128], Act.Square,
                                 scale=IS, accum_out=stats_cols[:, 2:3])
            nc.scalar.activation(junk[:, 0:256], T1[:, 128:384], Act.Square,
                                 scale=SQ2 * IS, accum_out=stats_cols[:, 3:4])
            # T2/T3 squares on the DVE pipe (psum -> sbuf copy, then STT)
            t2c = spool.tile([128, 256], f32, tag="t2c")
            nc.vector.tensor_copy(t2c[:], T2[:])
            jk2 = spool.tile([128, 256], f32, tag="jk2")
            nc.vector.scalar_tensor_tensor(
                jk2[:, 0:128], t2c[:, 0:128], IS * IS, t2c[:, 0:128],
                Alu.mult, Alu.mult, accum_out=stats_cols[:, 4:5])
            nc.vector.scalar_tensor_tensor(
                jk2[:, 128:256], t2c[:, 128:256], 2.0 * IS * IS, t2c[:, 128:256],
                Alu.mult, Alu.mult, accum_out=stats_cols[:, 5:6])
            t3c = spool.tile([128, 128], f32, tag="t3c")
            nc.vector.tensor_copy(t3c[:], T3[:])
            jk3 = spool.tile([128, 128], f32, tag="jk3")
            nc.vector.scalar_tensor_tensor(
                jk3[:], t3c[:], IS * IS, t3c[:],
                Alu.mult, Alu.mult, accum_out=stats_cols[:, 6:7])
            nc.vector.reduce_sum(stats_cols[:, 7:8], wsq_all[:], axis=AX.X)
            wsq_scr = spool.tile([128, KC], f32, tag="wsq_scr")
            nc.vector.scalar_tensor_tensor(
                wsq_scr[:], wsq_all[:], 1.0, wsq_all[:],
                Alu.mult, Alu.mult, accum_out=stats_cols[:, 8:9],
            )
            # s, u rows -> SBUF; u to partition 0 via DMA; dots into scr
            scr = sb.tile([1, 16], f32, tag="scr")
            su_row = sb.tile([2, 512], f32, tag="su_row")
            nc.vector.tensor_copy(su_row[:], SU[:])
            u_row = sb.tile([1, 512], f32, tag="u_row")
            nc.sync.dma_start(u_row[0:1, :], su_row[1:2, :])
            su_junk = spool.tile([1, 512], f32, tag="su_junk")
            nc.vector.scalar_tensor_tensor(
                su_junk[0:1, :], su_row[0:1, :], 1.0, su_row[0:1, :],
                Alu.mult, Alu.mult, accum_out=scr[0:1, 12:13],
            )
            nc.vector.scalar_tensor_tensor(
                su_junk[0:1, :], su_row[0:1, :], 1.0, u_row[0:1, :],
                Alu.mult, Alu.mult, accum_out=scr[0:1, 13:14],
            )
            # cross-partition reduce of the 9 partials in one fp32 ones-matmul
            ones_col = sb.tile([128, 1], f32, tag="ones_col")
            nc.vector.memset(ones_col[:], 1.0)
            cpres = T3[0:1, 16:25]
            nc.tensor.matmul(cpres, ones_col[:], stats_cols[:],
                             start=True, stop=True, skip_group_check=True)

            # scalar assembly on partition 0.  Raw scales:
            #   A_cp = 2^10 A, B_cp = 2^20 B, ssq_cp = 2^10 ssq,
            #   us_cp = 2^10 us (q = wsq, u_row = 32u), g2 = true ||G||^2.
            nc.vector.tensor_copy(scr[0:1, 0:9], cpres)
            g2s = scr[0:1, 9:10]
            nc.vector.reduce_sum(g2s, scr[0:1, 0:7], axis=AX.X)
            A_ap = scr[0:1, 7:8]
            B_ap = scr[0:1, 8:9]
            ssq_ap = scr[0:1, 12:13]
            us_ap = scr[0:1, 13:14]
            x_ap = scr[0:1, 10:11]
            t_ap = scr[0:1, 11:12]
            rw_ap = scr[0:1, 14:15]
            # X = 2^10*S1 = mc*A_cp - ssq_cp
            nc.vector.scalar_tensor_tensor(
                x_ap, A_ap, float(MC), ssq_ap, Alu.mult, Alu.subtract)
            # t = (X * 2^-10/T)^2 = mu^2
            nc.vector.tensor_scalar_mul(x_ap, x_ap, INV_T / 1024.0)
            nc.vector.tensor_mul(t_ap, x_ap, x_ap)
            # Y = 2^20*S2 = mc*B_cp + A_cp^2 + 2^21*g2 - 2^7*us_cp
            a2 = scr[0:1, 10:11]  # reuse
            nc.vector.tensor_mul(a2, A_ap, A_ap)
            nc.vector.scalar_tensor_tensor(
                rw_ap, B_ap, float(MC), a2, Alu.mult, Alu.add)
            nc.vector.tensor_scalar_mul(us_ap, us_ap, 4096.0)  # 2^12*us_cp
            nc.vector.scalar_tensor_tensor(
                a2, g2s, float(2.0 ** 21), us_ap, Alu.mult, Alu.subtract)
            nc.vector.tensor_add(rw_ap, rw_ap, a2)
            # rw = Y*2^-20/T - mu^2
            nc.vector.scalar_tensor_tensor(
                rw_ap, rw_ap, INV_T / (2.0 ** 20), t_ap, Alu.mult, Alu.subtract)
            nc.sync.dma_start(rw_out[0:1, 0:1], rw_ap)

    return nc


def _batch_perm():
    # device batch index d = bc*128 + p holds original row 4p + bc
    d = np.arange(B)
    p, bc = d % 128, d // 128
    return 4 * p + bc


def _prep_inputs(f, W):
    e4 = ml_dtypes.float8_e4m3
    f16 = np.float16
    f = np.asarray(f, dtype=np.float32)
    w_flat = np.asarray(W, dtype=np.float32).reshape(MC, H)

    W8_full = np.zeros((MC_PAD, WROW), dtype=e4)
    W8_full[:MC, :H] = (w_flat * WSCALE).astype(e4)
    W8_full[:, H] = e4(1.0)

    operm = _batch_perm()
    fh = f.astype(f16)
    fT_np = np.ascontiguousarray(fh[operm].T)  # column d = original 4p+bc
    fR_np = fh                                 # natural: (p bc) rows
    Wh = w_flat.astype(f16)
    z_W8 = np.zeros((MC_PAD, WROW), dtype=e4)
    z_DIN = np.zeros((128, 6400), dtype=f16)

    in_maps = [{"W8": W8_full, "DIN": z_DIN}]
    for k in range(1, N_CORES):
        r0 = SHARD_N * (k - 1)
        r1 = min(r0 + SHARD_N, MC)
        WT_np = np.zeros((H, SHARD_N), dtype=f16)
        WT_np[:, : r1 - r0] = Wh[r0:r1].T
        fT4 = fT_np.reshape(128, 4, B)
        WT4 = WT_np.reshape(128, 4, SHARD_N)
        fR4 = fR_np.reshape(128, 4, H)
        din = np.concatenate([
            fT4[:, 0:2].reshape(128, -1), WT4[:, 0:2].reshape(128, -1),
            fR4[:, 0].reshape(128, -1),
            fT4[:, 2:4].reshape(128, -1), WT4[:, 2:4].reshape(128, -1),
            fR4[:, 1].reshape(128, -1),
            fR4[:, 2:4].reshape(128, -1),
        ], axis=1)
        in_maps.append({"W8": z_W8, "DIN": np.ascontiguousarray(din)})
    return in_maps


def kernel(f, W, trace=False):
    from concourse.bass_utils import run_bass_kernel_spmd

    nc = _CACHE.get("nc")
    if nc is None:
        nc = _build()
        _CACHE["nc"] = nc

    in_maps = _prep_inputs(f, W)
    kwargs = {}
    if trace:
        kwargs["trace_cores"] = [0, 4]
    res = run_bass_kernel_spmd(
        nc, in_maps, core_ids=list(range(N_CORES)), trace=trace, **kwargs
    )
    _CACHE["last_result"] = res

    out = np.empty((B, NCLS + 1), dtype=np.float32)
    for k in range(1, N_CORES):
        c0 = SHARD_C * (k - 1)
        ncls = min(SHARD_C, NCLS - c0)
        if ncls <= 0:
            continue
        out[:, c0:c0 + ncls] = res.results[k]["dist"][:, :ncls]
    out[:, NCLS] = res.results[0]["rw"][0, 0]
    return out
